# revision 1
# baseline (speedup 1.0000x reference)
"""Distributed Bass attention kernel for 8 TRN2 NeuronCores.

Problem: full-dim attention (no head split), x:(2,4096,2048), 4x 2048^2 weights.

Sharding: batch+sequence parallel. Core c owns batch b=c//4 and query rows
[1024*(c%4), 1024*(c%4+1)). Each core computes its local K^T/V shard; the
shards are AllGather-ed within the 4-core replica group of the same batch.
Then each core computes transposed scores+softmax for its 8 q-tiles (sharing
each streamed K chunk), ctx^T (sharing each streamed V column chunk), and the
output projection. Host reassembles rows.

Perf notes baked in (measured ~1.12-1.13ms, ~7% under the 1.21ms baseline;
the PE is power-throttled to ~1.9 rows/ns sustained so the floor for this
matmul stream is ~1.12ms):
- All TensorE math bf16 (fp32 PSUM accumulate); rel err ~5e-3 (budget 2e-2;
  fp8 would give 3.6-5% -- barred).
- Scores are computed TRANSPOSED (lhsT=K-tile, rhs=q^T) so exp writes P^T
  directly -- no PE transposes, no PSUM->SBUF P copies. Softmax needs no max
  subtraction (scores ~ N(0,1)); row sums come from a ones-matmul over P^T,
  1/l applied by the DVE during the ctx^T PSUM evacuation.
- Every PE idle gap also costs a ~10us half-rate HAM cold window, so the
  whole schedule exists to keep the PE queue fed wall-to-wall:
  * x and the first k weight column arrive in small contiguous chunks
    ordered exactly as the first accumulation group consumes them -> first
    matmul ~15us (the first ~7us are fixed NEFF preamble).
  * Projection order K -> V -> Q. AG(K) is split into 4 e-row quarters
    (kt_b pre-tiled [p, q4, et%4, 256]) triggered at 25/50/75/100% of the
    k-projection -- all done ~100us before scores, and every K stream
    chunk load is 4 fully contiguous [128,1024] pieces (strided loads
    measured 29us of descriptor-gen under AG contention, blocking the
    ring behind them).
  * AG(V) is split into 4 d-column quarters, EACH dep-delayed onto a K
    stream chunk DMA (kbuf 10/12/14): the CC stream runs whichever
    collective's inputs are ready, so an undelayed quarter jumps the queue
    and its traffic starves the q-proj weight stream (measured 76us).
    Staggered anchors spread the quarters across the score phase.
  * wv weights + q-preload columns load via gpsimd SWDGE held behind the
    urgent k weights by an explicit dep (up-front they saturate HBM and
    starve the k columns; behind bounce stores they inherit compute
    gating -- both measured).
  * K stream chunks live in a right-side SBUF pool (bufs=10; addresses
    never overlap the projection working set); chunks 0-3 prefetch on the
    SYNC ring during q-proj (on scalar their AG-gated DMAs would block the
    q weight loads behind them).
  * V column chunks stream on the gpsimd SWDGE ring; wo quarters + V
    column buffers live in the right-side pool freed by the K stream (WAR
    clears at scores-end) so the output projection starts immediately
    after ctx.
  * k bounce stores split 2+2 across both HWDGE rings (64KB contiguous
    pieces); v bounce stores on gpsimd (on the rings their delay stalls
    the projection through staging-buffer reuse).
  * the last output store is split so the post-last-matmul tail is short.
"""

import numpy as np
import ml_dtypes

BF16 = ml_dtypes.bfloat16

D = 2048          # model dim
S = 4096          # sequence length per batch
BATCH = 2
NCORES = 8
GROUP = 4         # replica group size (cores per batch)
ROWS = S // GROUP  # query rows per core = 1024
P = 128           # partitions
DT = D // P       # 16 d-tiles
IT = ROWS // P    # 8 i-tiles per core
JT = S // P       # 32 j-tiles (full seq)
NCH = S // 512    # 8 key chunks
SCALE = 1.0 / float(np.sqrt(D))

_CACHE = {}


def _build():
    from concourse import bacc, mybir, tile
    from concourse.bass import _add_dep_helper

    f32 = mybir.dt.float32
    bf16 = mybir.dt.bfloat16

    nc = bacc.Bacc("TRN2", target_bir_lowering=False, debug=False,
                   num_devices=NCORES)

    # host-pre-tiled inputs (see _in_maps): every load is contiguous rows
    xt_d = nc.dram_tensor("xt", [2, 4, P, DT // 4 * 512], bf16,
                          kind="ExternalInput")
    wqt_d = nc.dram_tensor("wqt", [DT, P, DT * P], bf16, kind="ExternalInput")
    wkt_d = nc.dram_tensor("wkt", [DT, P, DT * P], bf16, kind="ExternalInput")
    wvt_d = nc.dram_tensor("wvt", [4, P, DT * 512], bf16, kind="ExternalInput")
    wot_d = nc.dram_tensor("wot", [4, P, DT * 512], bf16, kind="ExternalInput")
    out_d = nc.dram_tensor("out", [ROWS, D], f32, kind="ExternalOutput")

    RG = [[0, 1, 2, 3], [4, 5, 6, 7]]

    def all_gather(src, dst):
        return nc.gpsimd.collective_compute(
            "AllGather", mybir.AluOpType.bypass, replica_groups=RG,
            ins=[src.opt()], outs=[dst.opt()])

    with tile.TileContext(nc) as tc:
        dram = tc.alloc_tile_pool(name="dram", bufs=1, space="DRAM")
        persist = tc.alloc_tile_pool(name="persist", bufs=1)
        psum = tc.alloc_tile_pool(name="psum", bufs=2, space="PSUM")
        qtpool = tc.alloc_tile_pool(name="qtpool", bufs=1)

        # K^T bounce + gather buffers, split in e-row QUARTERS (piece q =
        # e-tiles 4q..4q+3) and stored pre-tiled [p, q4, et%4, 256]:
        # (a) piece q completes at (q+1)/4 of the et-major k-projection, so
        #     the 4 AllGathers pipeline behind it and all finish ~130us
        #     before the score phase -- AG(K) is off the critical path;
        # (b) every K stream chunk load is 4 fully contiguous [128, 1024]
        #     blocks -- strided HWDGE descriptor generation for these loads
        #     measured up to 29us under AG contention, blocking everything
        #     behind it on the issuing ring.
        kt_b = [dram.tile([P, 4 * 1024], bf16, name=f"kt_b{q}")
                for q in range(4)]
        kt_g = [dram.tile([GROUP, P, 4 * 1024], bf16, name=f"kt_g{q}")
                for q in range(4)]
        # V bounce/gather in d-column QUARTERS: AG(V) runs as four small
        # pipelined pieces so the ctx phase (which consumes d'-tiles in
        # order) never waits for the full gather
        v_b = [dram.tile([ROWS, 512], bf16, name=f"v_b{h}")
               for h in range(4)]
        v_g = [dram.tile([GROUP, ROWS, 512], bf16, name=f"v_g{h}")
               for h in range(4)]

        linv_bc = persist.tile([P, ROWS], f32)  # 1/l bcast on partitions
        ones = persist.tile([P, P], bf16)

        # q^T [e, i] during proj+scores; ctx^T [d', i] afterwards
        qt_s = qtpool.tile([P, DT, ROWS], bf16)
        ctxt_s = qt_s

        # ---------------- Phase 1: projections ----------------
        proj = tc.alloc_tile_pool(name="proj", bufs=2)

        # warm both HWDGE rings so the first real loads skip the
        # first-DMA spin-up latency
        warm = proj.tile([P, 16], bf16, bufs=1)
        nc.sync.dma_start(out=warm[0:1, :], in_=xt_d[0, 0, 0:1, 0:16])
        nc.scalar.dma_start(out=warm[1:2, :], in_=xt_d[0, 0, 1:2, 0:16])
        nc.gpsimd.memset(ones[:], 1.0)

        # first k-proj weight column in 4 chunks so the first matmul gates
        # on 128KB
        pre_k0 = proj.tile([P, DT, P], bf16, tag="wcol", bufs=8,
                           name="wcol_pre_k0")
        nc.scalar.dma_start(out=pre_k0[:, 0:2, :], in_=wkt_d[0][:, 0:2 * P])
        nc.scalar.dma_start(out=pre_k0[:, 2:4, :],
                            in_=wkt_d[0][:, 2 * P:4 * P])
        for g in range(1, 4):
            nc.scalar.dma_start(
                out=pre_k0[:, 4 * g:4 * g + 4, :],
                in_=wkt_d[0][:, 4 * g * P:(4 * g + 4) * P])

        # x^T in SBUF as [p, c-half, dt, i], loaded in 8 contiguous 512KB
        # chunks ordered exactly as the first accumulation group consumes
        # them (c=0 dt-major on sync; c=1 on scalar, interleaved with the
        # first k weight columns so neither starves the other)
        xt_s = proj.tile([P, 2, DT, 512], bf16, bufs=1)
        nc.sync.dma_start(out=xt_s[:, 0, 0:2, :], in_=xt_d[0, 0][:, 0:1024])
        nc.sync.dma_start(out=xt_s[:, 0, 2:4, :],
                          in_=xt_d[0, 0][:, 1024:2048])
        for g in range(1, 4):
            nc.sync.dma_start(out=xt_s[:, 0, 4 * g:4 * g + 4, :],
                              in_=xt_d[0, g])

        def xc1_load(g):
            nc.scalar.dma_start(out=xt_s[:, 1, 4 * g:4 * g + 4, :],
                                in_=xt_d[1, g])

        # k weight columns 1-3 interleave with the x c=1 chunks (both are
        # needed in the first ~30us); the rest stream bufs-paced
        wcols_k = [pre_k0]
        xc1_load(0)
        wk3_dma = None
        for et in range(1, 4):
            w = proj.tile([P, DT, P], bf16, tag="wcol", bufs=8)
            wk3_dma = nc.scalar.dma_start(out=w[:], in_=wkt_d[et])
            wcols_k.append(w)
            xc1_load(et)

        # ALL v-proj weights + the first two q-proj columns load on the
        # gpsimd SWDGE ring, held behind the urgent k weights via an
        # explicit dep: issued on sync/scalar they either sit behind
        # compute-gated bounce stores (measured 40us v-proj stall) or
        # saturate HBM exactly when the first k weight columns are needed
        # (measured ~15us of early k-proj stalls)
        projv = tc.alloc_tile_pool(name="projv", bufs=2)
        wvqs = []
        for qd in range(4):
            wvq = projv.tile([P, DT, 512], bf16, tag="wq4", bufs=4)
            wdma = nc.gpsimd.dma_start(out=wvq[:], in_=wvt_d[qd])
            if qd == 0:
                _add_dep_helper(wdma.ins, wk3_dma.ins, sync=True,
                                reason="hold v weights behind k weights")
            wvqs.append(wvq)
        # own tag: these are consumed only at q-proj start, so they must
        # NOT share the k-weight slot rotation (slot-reuse dependency
        # cycle with the in-order PE queue)
        pre_q = []
        for et in range(2):
            wcol = proj.tile([P, DT, P], bf16, tag="wcolq",
                             bufs=2, name=f"wcol_pre{et}")
            nc.gpsimd.dma_start(out=wcol[:], in_=wqt_d[et])
            pre_q.append(wcol)

        # k-proj, e-tile major; piece q of the bounce buffer completes at
        # (q+1)/4 of the projection and its AllGather triggers right away
        # (except piece 3, whose trigger is deferred into the v-projection
        # so its input-store wait cannot block the v bounce stores on the
        # in-order gpsimd queue -- measured 12us stall)
        for et in range(DT):
            if et < 4:
                wcol = wcols_k[et]
            else:
                wcol = proj.tile([P, DT, P], bf16, tag="wcol", bufs=8)
                nc.scalar.dma_start(out=wcol[:], in_=wkt_d[et])
            kt_t = proj.tile([P, ROWS], bf16, tag="kt_t", bufs=8)
            for c in range(2):
                ps = psum.tile([P, 512], f32, tag="acc")
                for dt_i in range(DT):
                    nc.tensor.matmul(
                        ps[:],
                        wcol[:, dt_i, :],
                        xt_s[:, c, dt_i, :],
                        start=(dt_i == 0),
                        stop=(dt_i == DT - 1))
                nc.vector.tensor_copy(kt_t[:, c * 512:(c + 1) * 512], ps[:])
            for q4 in range(4):  # 64KB contiguous stores, 2 per ring
                (nc.sync if q4 % 2 == 0 else nc.scalar).dma_start(
                    out=kt_b[et // 4][:, q4 * 1024 + (et % 4) * 256:
                                      q4 * 1024 + (et % 4) * 256 + 256],
                    in_=kt_t[:, q4 * 256:(q4 + 1) * 256])
            if et % 4 == 3 and et != DT - 1:
                all_gather(kt_b[et // 4], kt_g[et // 4])

        def kbuf_load(kbuf, cidx, eng):
            # 4 fully contiguous [128, 1024] piece loads per chunk
            r, q4 = cidx // GROUP, cidx % GROUP
            dmas = []
            for q in range(4):
                dmas.append(eng.dma_start(
                    out=kbuf[:, 4 * q:4 * q + 4, :],
                    in_=kt_g[q][r, :, q4 * 1024:(q4 + 1) * 1024]))
            return dmas


        # v: out[j-tile, d'] = sum_d x^T[d,j]^T wvt[d,d']
        # Runs LAST among the projections: its weights are fully
        # preloaded, and each AG(V) quarter triggers right behind its
        # v-proj quarter, pipelining the V gather with the projection --
        # all quarters land long before the ctx phase consumes them, with
        # no delay hacks. Trigger placement keeps any
        # wait-for-previous-collective off the path of the bounce stores.
        for qd in range(4):
            wvq = wvqs[qd]
            v_stage = projv.tile([P, IT, 512], bf16, tag="v_stage", bufs=2)
            for jt in range(IT):
                ps = psum.tile([P, 512], f32, tag="acc")
                for dt_i in range(DT):
                    nc.tensor.matmul(
                        ps[:],
                        xt_s[:, jt // 4, dt_i,
                             (jt % 4) * P:(jt % 4 + 1) * P],
                        wvq[:, dt_i, :],
                        start=(dt_i == 0),
                        stop=(dt_i == DT - 1))
                nc.vector.tensor_copy(v_stage[:, jt, :], ps[:])
            nc.gpsimd.dma_start(
                out=v_b[qd][:]
                .rearrange("(jt p) d -> p jt d", p=P),
                in_=v_stage[:])
            if qd == 1:
                # deferred AG(K) piece 3: its input-store wait must not
                # block the v bounce stores on the in-order gpsimd queue
                # (piece-3 stores complete late under AG contention)
                all_gather(kt_b[3], kt_g[3])
        # ALL V-quarter gathers are held behind the K-stream anchor (the
        # CC stream runs whichever collective's inputs are ready, so an
        # undelayed quarter jumps the queue and contends with the q-proj
        # weight loads -- measured 76us starvation)
        projv.release()

        # K stream chunks live on the RIGHT side of SBUF (addresses never
        # overlap the projection working set) and prefetch on the SYNC
        # ring during the q-projection, gated only by AG(K): on the scalar
        # ring their AG-waiting piece DMAs would block every q-proj weight
        # load behind them (measured 54us q-proj starvation)
        kpool = tc.alloc_tile_pool(name="kpool", bufs=1, side="right")
        kbufs = []
        for cidx in range(4):
            kbuf = kpool.tile([P, DT, 256], bf16, tag="kbuf", bufs=10)
            kbuf_load(kbuf, cidx, nc.sync)
            kbufs.append(kbuf)

        # q-proj
        qcopy = None
        for et in range(DT):
            if et < len(pre_q):
                wcol = pre_q[et]
            else:
                wcol = proj.tile([P, DT, P], bf16, tag="wcol", bufs=8)
                nc.scalar.dma_start(out=wcol[:], in_=wqt_d[et])
            for c in range(2):
                ps = psum.tile([P, 512], f32, tag="acc")
                for dt_i in range(DT):
                    nc.tensor.matmul(
                        ps[:],
                        wcol[:, dt_i, :],
                        xt_s[:, c, dt_i, :],
                        start=(dt_i == 0),
                        stop=(dt_i == DT - 1))
                qcopy = nc.vector.tensor_copy(
                    qt_s[:, et, c * 512:(c + 1) * 512], ps[:])

        proj.release()

        # ------------- Phase 2: attention -------------
        attn = tc.alloc_tile_pool(name="attn", bufs=2)
        pt_s = attn.tile([P, JT, IT * P], bf16, bufs=1)
        # ALL V-quarter gathers are dep-delayed (the CC stream runs
        # whichever collective's inputs are ready, so an undelayed quarter
        # jumps the queue and contends with the q-proj weight stream --
        # measured 76us starvation). The first ctx V-column buffer is
        # issued between the trigger pairs: it loads right after quarter 0
        # lands, ~200us before the ctx phase needs it, and is not blocked
        # by the later triggers' wait-for-previous-collective.
        cc_vs = [all_gather(v_b[h], v_g[h]) for h in range(2)]
        vcol0 = attn.tile([P, NCH, 4, 256], bf16, tag="vcol0", bufs=1)
        for g in range(NCH):
            r, h = g // 2, g % 2
            nc.gpsimd.dma_start(
                out=vcol0[:, g, :, :],
                in_=v_g[0][r, h * 512:(h + 1) * 512, 0:256]
                .rearrange("(t p) d -> p t d", p=P))
        cc_vs += [all_gather(v_b[h], v_g[h]) for h in range(2, 4)]
        # --- A: transposed scores + exp, all 8 i-tiles ---
        for cidx in range(2 * NCH):  # 16 chunks of 256 keys
            if cidx < 4:
                kbuf = kbufs[cidx]
            else:
                kbuf = kpool.tile([P, DT, 256], bf16, tag="kbuf", bufs=10)
                # alternate rings: two parallel HWDGE FIFOs keep the K
                # stream ahead of compute while AG(V)'s SDMA traffic
                # slows individual transfers
                eng = nc.sync if cidx % 2 == 0 else nc.scalar
                kdma = kbuf_load(kbuf, cidx, eng)[0]
                # These DMAs complete early in the score phase (slot WAR
                # on chunks 0/2/4): staggered anchors for the V-gather
                # delays, spreading the four quarter-gathers across the
                # score phase so at most one contends the K stream at a
                # time.
                anchor = {10: (0, 1), 12: (2,), 14: (3,)}.get(cidx, ())
                for h in anchor:
                    _add_dep_helper(
                        cc_vs[h].ins, kdma.ins, sync=True,
                        reason="delay AG(V) past q-proj/K stream")
            for jl in range(2):
                jt = cidx * 2 + jl
                for ib in range(2):
                    sps = psum.tile([P, 512], f32, tag="scores", bufs=4)
                    for e in range(DT):
                        nc.tensor.matmul(
                            sps[:],
                            kbuf[:, e, jl * P:(jl + 1) * P],
                            qt_s[:, e, ib * 512:(ib + 1) * 512],
                            start=(e == 0),
                            stop=(e == DT - 1))
                    nc.scalar.activation(
                        pt_s[:, jt, ib * 512:(ib + 1) * 512],
                        sps[:],
                        mybir.ActivationFunctionType.Exp,
                        scale=SCALE)
        kpool.release()

        # wo quarters prefetch into the space freed by the K stream; the
        # WAR dependency clears at scores-end so they all land during ctx
        # and the output projection starts immediately after ctx.
        oproj = tc.alloc_tile_pool(name="oproj", bufs=2, side="right")
        woqs = []
        for fq in range(4):
            woq = oproj.tile([P, DT, 512], bf16, tag="woq", bufs=3)
            nc.scalar.dma_start(out=woq[:], in_=wot_d[fq])
            woqs.append(woq)

        # --- rowsums via ones-matmul: l bcast on partitions
        for ib in range(2):
            lps = psum.tile([P, 512], f32, tag="ctx")
            for jt in range(JT):
                nc.tensor.matmul(
                    lps[:], ones[:],
                    pt_s[:, jt, ib * 512:(ib + 1) * 512],
                    start=(jt == 0), stop=(jt == JT - 1))
            nc.vector.reciprocal(
                linv_bc[:, ib * 512:(ib + 1) * 512], lps[:])

        # --- B: ctx^T[d', i] = sum_j V[j,d']^T P^T[j, i],
        #     scaled by 1/l during PSUM evacuation.
        #     V column chunks stream on the gpsimd SWDGE ring (third DMA
        #     issue path, idle engine) so they never queue behind the K
        #     stream on the HWDGE rings. The remaining AG(V) quarter
        #     triggers interleave so that a trigger's wait-for-previous-
        #     collective never blocks a vcol load that is needed sooner.
        for dp2 in range(DT // 2):  # pairs of d'-tiles
            if dp2 == 0:
                vcol = vcol0
            else:
                vcol = oproj.tile([P, NCH, 4, 256], bf16, tag="vcol",
                                  bufs=2)
                for g in range(NCH):  # j-block [512g, 512g+512)
                    r, h = g // 2, g % 2
                    nc.gpsimd.dma_start(
                        out=vcol[:, g, :, :],
                        in_=v_g[dp2 // 2][r, h * 512:(h + 1) * 512,
                                          (dp2 % 2) * 256:
                                          (dp2 % 2) * 256 + 256]
                        .rearrange("(t p) d -> p t d", p=P))
            for ds in range(2):
                dp = dp2 * 2 + ds
                for ih in range(2):  # i-halves of 512
                    cps = psum.tile([P, 512], f32, tag="ctx")
                    for jt in range(JT):
                        nc.tensor.matmul(
                            cps[:],
                            vcol[:, jt // 4, jt % 4,
                                 ds * P:(ds + 1) * P],
                            pt_s[:, jt, ih * 512:(ih + 1) * 512],
                            start=(jt == 0),
                            stop=(jt == JT - 1))
                    nc.vector.tensor_tensor(
                        out=ctxt_s[:, dp, ih * 512:(ih + 1) * 512],
                        in0=cps[:],
                        in1=linv_bc[:, ih * 512:(ih + 1) * 512],
                        op=mybir.AluOpType.mult)
        attn.release()

        # ------------- Phase 3: output projection -------------
        osbp = tc.alloc_tile_pool(name="osbp", bufs=2)
        for fq in range(4):
            woq = woqs[fq]
            osb = osbp.tile([P, IT, 512], f32, tag="osb", bufs=2)
            for it in range(IT):
                ops = psum.tile([P, 512], f32, tag="acc")
                for dp in range(DT):
                    nc.tensor.matmul(
                        ops[:],
                        ctxt_s[:, dp, it * P:(it + 1) * P],
                        woq[:, dp, :],
                        start=(dp == 0), stop=(dp == DT - 1))
                nc.scalar.copy(osb[:, it, :], ops[:])
            # merged stores; the final quarter is split so the tail after
            # the last matmul is a short 0.5MB store
            bounds = ((0, 8),) if fq < 3 else ((0, 4), (4, 6), (6, 7), (7, 8))
            for lo, hi in bounds:
                nc.sync.dma_start(
                    out=out_d[lo * P:hi * P, fq * 512:(fq + 1) * 512]
                    .rearrange("(it p) f -> p it f", p=P),
                    in_=osb[:, lo:hi, :])
        osbp.release()
        oproj.release()
        qtpool.release()
        persist.release()
        psum.release()
        dram.release()

    nc.compile()
    return nc


def _get_nc():
    if "nc" not in _CACHE:
        _CACHE["nc"] = _build()
    return _CACHE["nc"]


def _tile_we(w):
    # [out,in] weight -> w.T tiled as [et, p, dt*128] contiguous
    wt = np.ascontiguousarray(np.asarray(w, np.float32).T)  # [d, e]
    t = wt.reshape(DT, P, DT, P).transpose(2, 1, 0, 3)      # [et, p, dt, e]
    return np.ascontiguousarray(t.reshape(DT, P, DT * P)).astype(BF16)


def _tile_wq4(w):
    # [out,in] weight -> w.T tiled as [qd, p, dt*512] contiguous
    wt = np.ascontiguousarray(np.asarray(w, np.float32).T)  # [d, dcol]
    t = wt.reshape(DT, P, 4, 512).transpose(2, 1, 0, 3)     # [qd, p, dt, dc]
    return np.ascontiguousarray(t.reshape(4, P, DT * 512)).astype(BF16)


def _in_maps(x, wq, wk, wv, wo):
    wqt = _tile_we(wq)
    wkt = _tile_we(wk)
    wvt = _tile_wq4(wv)
    wot = _tile_wq4(wo)
    x = np.asarray(x, np.float32)
    maps = []
    for c in range(NCORES):
        b, r = c // GROUP, c % GROUP
        xt = x[b, r * ROWS:(r + 1) * ROWS, :].T          # [d, i]
        # -> [c-half, dt-group-of-4, p, dt-in-group*512] contiguous chunks
        xt = xt.reshape(4, 4, P, 2, 512).transpose(3, 0, 2, 1, 4)
        xt = np.ascontiguousarray(xt.reshape(2, 4, P, 2048)).astype(BF16)
        maps.append({"xt": xt, "wqt": wqt, "wkt": wkt, "wvt": wvt,
                     "wot": wot})
    return maps


def run(x, wq, wk, wv, wo, trace=False, **trace_kwargs):
    from concourse.bass_utils import run_bass_kernel_spmd
    nc = _get_nc()
    res = run_bass_kernel_spmd(nc, _in_maps(x, wq, wk, wv, wo),
                               list(range(NCORES)), trace=trace,
                               **trace_kwargs)
    out = np.empty((BATCH, S, D), np.float32)
    for c in range(NCORES):
        b, r = c // GROUP, c % GROUP
        out[b, r * ROWS:(r + 1) * ROWS, :] = res.results[c]["out"]
    return out, res


def kernel(x, wq, wk, wv, wo):
    out, _ = run(x, wq, wk, wv, wo)
    return out

